# revision 42
# baseline (speedup 1.0000x reference)
"""Multi-head attention (B=2, S=2048, D=1024, H=16) on 8 TRN2 NeuronCores.

Sharding: tensor-parallel over heads. Core c owns heads {2c, 2c+1}:
  - Q/K/V projections for its 128 feature columns (fp16 weights/x, fp32r Q/K),
  - attention for its 2 heads over both batches; exp'd scores (bf16) act as
    the matmul *stationary* so attention-output lands token-major with the
    softmax denominator accumulated via a ones-column of V,
  - four AllToAll pieces (one per batch x head-slot, bf16, 256KB) convert
    head-sharding -> token-sharding; each core owns 256 tokens of EACH batch
    so batch-0 output projection overlaps batch-1 attention,
  - output projection (full Wo, bf16) per batch for its token slices.
The Activation engine (exp of 16.8M scores/core) is the critical resource:
emission keeps its score->exp pipeline fed from ~16us on, while AV matmuls,
projections for later tiles, V transposes, receive transposes and the
out-projection all flow through a slot-budgeted filler/pend scheduler into
the Act-paced gaps of the strictly in-order PE stream.
Host only reshapes/transposes/concatenates.
"""
import sys
sys.path.insert(0, "/opt/trn_rl_repo")
from collections import deque
from contextlib import ExitStack

import numpy as np

import concourse.bass as bass
import concourse.bacc as bacc
import concourse.mybir as mybir
import concourse.tile as tile
from concourse.bass_utils import run_bass_kernel_spmd

N_CORES = 8
B, S, D = 2, 2048, 1024
T = B * S              # 4096 flattened tokens
H, DH = 16, 64
F = D // N_CORES       # 128 feature columns per core (2 heads)
ND = D // 128          # 8 contraction chunks
NKT = S // 128         # 16 key tiles per batch
OWN = S // N_CORES     # 256 tokens owned per batch per core
BACKLOG = 8            # target AV-lag (iterations) behind the score stream
LAG_MIN = 2            # never let AV catch up closer than this

F32 = mybir.dt.float32
F32R = mybir.dt.float32r
BF16 = mybir.dt.bfloat16
FP16 = mybir.dt.float16
EXP = mybir.ActivationFunctionType.Exp

_cache = {}


def build_nc():
    nc = bacc.Bacc()
    xT_e = nc.dram_tensor("xT", [D, T], FP16, kind="ExternalInput")
    wq_e = nc.dram_tensor("wq", [128, D], FP16, kind="ExternalInput")
    wk_e = nc.dram_tensor("wk", [128, D], FP16, kind="ExternalInput")
    wv_e = nc.dram_tensor("wv", [128, D], FP16, kind="ExternalInput")
    bq_e = nc.dram_tensor("bq", [F, 1], F32, kind="ExternalInput")
    bk_e = nc.dram_tensor("bk", [F, 1], F32, kind="ExternalInput")
    bv_e = nc.dram_tensor("bv", [F, 1], F32, kind="ExternalInput")
    wo_e = nc.dram_tensor("wo", [128, ND * D], BF16, kind="ExternalInput")
    bo_e = nc.dram_tensor("bo", [128, ND], F32, kind="ExternalInput")
    id_e = nc.dram_tensor("ident", [128, 128], BF16, kind="ExternalInput")
    outT_e = nc.dram_tensor("outT", [D, 2 * OWN], F32, kind="ExternalOutput")
    dbg = {}
    if _cache.get("debug"):
        for nm, shape, dt in (("dQt", [F, T], F32R), ("dKt", [F, T], F32R),
                              ("dVt", [F, T], BF16), ("dOT0", [128, 1024], BF16),
                              ("dOT1", [128, 1024], BF16),
                              ("dof0", [128, ND * 256], BF16)):
            dbg[nm] = nc.dram_tensor(nm, shape, dt, kind="ExternalOutput")

    with tile.TileContext(nc) as tc, ExitStack() as top:
        # persistent SBUF tensors
        big = top.enter_context(tc.tile_pool(name="big", bufs=1))
        Qt = big.tile([F, T], F32R, tag="Qt")        # [feat, tok]
        Kt = big.tile([F, T], F32R, tag="Kt")
        Vt = big.tile([F, T], BF16, tag="Vt")
        w_sb = {n: big.tile([128, D], FP16, tag=f"w{n}", name=f"w_{n}")
                for n in ("q", "k", "v")}
        wo_sb = big.tile([128, ND * D], BF16, tag="wo")

        misc = top.enter_context(tc.tile_pool(name="misc", bufs=1))
        bq_sb = misc.tile([F, 1], F32)
        bk_sb = misc.tile([F, 1], F32)
        bv_sb = misc.tile([F, 1], F32)
        bo_sb = misc.tile([128, ND], F32)
        id_sb = misc.tile([128, 128], BF16)

        xsp = top.enter_context(tc.tile_pool(name="xst", bufs=4))
        xs_tiles = {}

        # startup DMA order: first x tiles + wk first so K matmuls start ASAP
        for t in range(4):
            xs_tiles[t] = xsp.tile([128, ND * 512], FP16, tag="x", name=f"xst{t}")
        for t, (we, wn, be, bs) in enumerate((
                (wk_e, "k", bk_e, bk_sb), (wq_e, "q", bq_e, bq_sb),
                (wv_e, "v", bv_e, bv_sb), (None, None, bo_e, bo_sb))):
            xs = xs_tiles[t]
            nc.sync.dma_start(
                out=xs[:].rearrange("p (c f) -> p c f", c=ND)[:, :, 0:256],
                in_=xT_e[:, 512 * t:512 * t + 256].rearrange("(c p) f -> p c f", p=128))
            if we is not None:
                nc.scalar.dma_start(out=w_sb[wn][:], in_=we[:])
            nc.scalar.dma_start(
                out=xs[:].rearrange("p (c f) -> p c f", c=ND)[:, :, 256:512],
                in_=xT_e[:, 512 * t + 256:512 * (t + 1)].rearrange("(c p) f -> p c f", p=128))
            nc.sync.dma_start(out=bs[:], in_=be[:])
        nc.sync.dma_start(out=id_sb[:], in_=id_e[:])

        vsb = top.enter_context(tc.tile_pool(name="vsb", bufs=1))
        v_tiles = {}

        # OT[b]: normalized attn out, feat-major: rows 64h:64h+64 = slot h,
        # cols 128*qs + tok for the 16 q-slices
        onp = top.enter_context(tc.tile_pool(name="onp", bufs=1))
        # OT[b,h]: [64*(qs%2)+f, 128*(qs//2)+t] -- row-groups by query parity,
        # columns grouped by owning core
        OT = {(b, h): onp.tile([128, NKT * 64], BF16, tag=f"OT{b}{h}",
                               name=f"OT{b}{h}")
              for b in range(B) for h in range(2)}
        onrmp = top.enter_context(tc.tile_pool(name="onrm", bufs=20))

        dram = top.enter_context(tc.tile_pool(name="dram", bufs=1, space="DRAM"))
        pin = {}
        pout = {}
        for b in range(B):
            for h in range(2):
                pin[b, h] = dram.tile([N_CORES, DH, OWN], BF16, tag=f"pi{b}{h}",
                                      name=f"pi{b}{h}")
                pout[b, h] = dram.tile([N_CORES, DH, OWN], BF16, tag=f"po{b}{h}",
                                       name=f"po{b}{h}")

        ofp = top.enter_context(tc.tile_pool(name="ofp", bufs=1))
        of = {b: ofp.tile([128, ND * 256], BF16, tag=f"of{b}", name=f"of{b}")
              for b in range(B)}
        osbp = top.enter_context(tc.tile_pool(name="osbp", bufs=1))
        osb = {b: osbp.tile([128, ND * OWN], F32, tag=f"osb{b}", name=f"osb{b}")
               for b in range(B)}

        # shared 1-bank transpose pool, alive for the whole kernel
        trp = top.enter_context(tc.tile_pool(name="trp", bufs=1, space="PSUM"))

        # --- emission helpers: items are closures (one PE instruction each,
        # DVE followups attached), consumed via the slot scheduler ---
        def proj_unit_items(psp, proj, t, dst, bias_sb):
            cell = []
            for dk in range(ND):
                def mm(dk=dk, cell=cell, proj=proj, t=t, psp=psp,
                       dst=dst, bias_sb=bias_sb):
                    if dk == 0:
                        cell.append(psp.tile([128, 512], F32, tag="pj",
                                             name=f"pj_{proj}{t}"))
                    ps = cell[0]
                    nc.tensor.matmul(ps[:], w_sb[proj][:, 128 * dk:128 * (dk + 1)],
                                     xs_tiles[t][:, 512 * dk:512 * (dk + 1)],
                                     start=(dk == 0), stop=(dk == ND - 1))
                    if dk == ND - 1:
                        sl = slice(512 * t, 512 * (t + 1))
                        nc.vector.tensor_scalar_add(dst[:, sl], ps[:], bias_sb[:])
                yield mm

        def vtr_items(pool, t):
            b = t // 4
            for i in range(4):
                kt = 4 * (t % 4) + i
                tok = 512 * t + 128 * i
                tp = pool.tile([128, 128], BF16, tag="tr", name=f"tr{t}{i}")

                def tr(tp=tp, tok=tok, b=b, kt=kt):
                    nc.tensor.transpose(tp[:], Vt[:, tok:tok + 128], id_sb[:])
                    for h in range(2):
                        vt = vsb.tile([128, 65], BF16, tag=f"v{b}{h}{kt}",
                                      name=f"v{b}{h}{kt}")
                        nc.vector.tensor_copy(vt[:, 0:64], tp[:, 64 * h:64 * (h + 1)])
                        nc.vector.memset(vt[:, 64:65], 1.0)
                        v_tiles[b, h, kt] = vt
                yield tr

        def run_items(items):
            for it in items:
                it()

        # ---- upfront: K(t0,t1), Q(t0,t1), V(t0)+vtr(t0) ----
        with ExitStack() as phA:
            pspA = phA.enter_context(tc.tile_pool(name="pspA", bufs=2, space="PSUM"))
            for t in range(2):
                run_items(proj_unit_items(pspA, "k", t, Kt, bk_sb))
            for t in range(2):
                run_items(proj_unit_items(pspA, "q", t, Qt, bq_sb))
            run_items(proj_unit_items(pspA, "v", 0, Vt, bv_sb))
            run_items(vtr_items(trp, 0))

        # ---- attention unit (b, h): scores/exp stream paces Act; AV, norm,
        # ship and filler flow through the pend/filler slot scheduler ----
        def emittable(fn):
            key = getattr(fn, "vt_key", None)
            return key is None or key in v_tiles

        def gated(fn):
            g = getattr(fn, "gate", None)
            return g is None or g()

        def attn_bh(b, h, scp, opsp, nrmp, atp, pend):
            unit_items = []
            hs = slice(64 * h, 64 * (h + 1))
            for qh in range(2):
                o_ps = [opsp.tile([128, 512], F32, tag=f"o{i}", bufs=1,
                                  name=f"ops{b}{h}{qh}{i}") for i in range(2)]
                for i in range(2):
                    nc.vector.memset(o_ps[i][:], 0.0)
                for kt in range(NKT):
                    ktok = 2048 * b + 128 * kt
                    sc = scp.tile([128, 1024], F32, tag="sc",
                                  name=f"sc{b}{h}{qh}{kt}")
                    for i in range(2):
                        qtok = 2048 * b + 1024 * qh + 512 * i
                        nc.tensor.matmul(sc[:, 512 * i:512 * (i + 1)],
                                         Kt[hs, ktok:ktok + 128],
                                         Qt[hs, qtok:qtok + 512],
                                         start=True, stop=True)
                    at = atp.tile([128, 1024], BF16, tag="at",
                                  name=f"at{b}{h}{qh}{kt}")
                    nc.scalar.activation(at[:], sc[:], EXP)

                    def avs(kt=kt, at=at, o_ps=o_ps, b=b, h=h):
                        vt = v_tiles[b, h, kt]
                        for j8 in range(8):
                            ti, j = j8 // 4, j8 % 4
                            nc.tensor.matmul(o_ps[ti][:, 128 * j:128 * j + 65],
                                             at[:, 128 * j8:128 * (j8 + 1)],
                                             vt[:, 0:65],
                                             start=False, stop=(kt == NKT - 1),
                                             skip_group_check=True)
                    avs.vt_key = (b, h, kt)
                    pend.append(avs)
                    it_ctr[0] += 1
                    # non-PE pend items (norm/ship/rec-DMA) pop for free
                    while pend and getattr(pend[0], "free", False):
                        pend.popleft()()
                    # slot budget: ~3 x 213ns of PE work besides the scores
                    spent = 0
                    while spent < 2 and len(pend) > BACKLOG and emittable(pend[0]):
                        it = pend.popleft()
                        spent += getattr(it, "slots", 1)
                        it()
                    while spent < 3:
                        if fill_hi and gated(fill_hi[0]):
                            it = fill_hi.popleft()
                        elif fill_lo and gated(fill_lo[0]):
                            it = fill_lo.popleft()
                        elif fill_bg and gated(fill_bg[0]):
                            it = fill_bg.popleft()
                        else:
                            break
                        spent += getattr(it, "slots", 1)
                        it()
                    while spent < 3 and len(pend) > LAG_MIN and emittable(pend[0]):
                        it = pend.popleft()
                        spent += getattr(it, "slots", 1)
                        it()

                onrm = {}

                def norm(o_ps=o_ps, qh=qh, b=b, h=h, onrm=onrm):
                    for ti in range(2):
                        rcp = nrmp.tile([128, 4], F32, tag="rcp",
                                        name=f"rcp{b}{h}{qh}{ti}")
                        den = o_ps[ti][:].rearrange("p (j c) -> p j c", c=128)[:, :, 64:65]
                        nc.vector.reciprocal(rcp[:].rearrange("p (j c) -> p j c", c=1), den)
                        for j in range(4):
                            qs = 8 * qh + 4 * ti + j
                            pair = (qs % 8) // 2
                            if pair not in onrm:
                                onrm[pair] = onrmp.tile([128, 128], BF16, tag="onrm",
                                                        name=f"onrm{b}{h}{qh}{pair}")
                            nc.vector.tensor_scalar_mul(
                                onrm[pair][:, 64 * (qs % 2):64 * (qs % 2 + 1)],
                                o_ps[ti][:, 128 * j:128 * j + 64],
                                rcp[:, j:j + 1])
                norm.free = True
                pend.append(norm)
                for pair in range(4):
                    def trc(pair=pair, qh=qh, b=b, h=h, onrm=onrm):
                        pg = 4 * qh + pair
                        tp = trp.tile([128, 128], BF16, tag="tr",
                                      name=f"otr{b}{h}{pg}")
                        nc.tensor.transpose(tp[:], onrm[pair][:], id_sb[:])
                        nc.vector.tensor_copy(
                            OT[b, h][:, 128 * pg:128 * (pg + 1)], tp[:])
                    trc.slots = 2
                    trc.gate = (lambda pair=pair, onrm=onrm: pair in onrm)
                    unit_items.append(trc)

            def ship(b=b, h=h):
                for j in range(2):
                    nc.sync.dma_start(
                        out=pin[b, h][:, :, 128 * j:128 * (j + 1)]
                        .rearrange("r f t -> f r t"),
                        in_=OT[b, h][64 * j:64 * (j + 1), :]
                        .rearrange("f (r t) -> f r t", r=N_CORES))
                nc.gpsimd.collective_compute(
                    "AllToAll", mybir.AluOpType.bypass,
                    ins=[pin[b, h][:].opt()], outs=[pout[b, h][:].opt()],
                    replica_groups=[list(range(N_CORES))])
            unit_items.append(ship)
            return unit_items

        rec_flags = {}
        it_ctr = [0]
        fill_hi = deque()
        fill_lo = deque()
        fill_bg = deque()

        def rec_dma(b, h, eng, half=None):
            def rdma(b=b, h=h, eng=eng, half=half):
                rsl = slice(0, N_CORES) if half is None else \
                    slice(4 * half, 4 * (half + 1))
                eng.dma_start(
                    out=of[b][64 * h:64 * (h + 1), :].rearrange(
                        "f (r t) -> f r t", r=N_CORES)[:, rsl, :],
                    in_=pout[b, h][rsl].rearrange("r f t -> f r t"),
                )
                rec_flags[b, h] = it_ctr[0]
            return rdma

        def after_rec(b, h, margin):
            """Gate: open `margin` iterations after the (b,h) receive DMA was
            dispatched (collective + transfer latency, in ~1.05us iterations)."""
            def g():
                it = rec_flags.get((b, h))
                return it is not None and it_ctr[0] >= it + margin
            return g


        def outproj_items(b, outp, n_range=range(ND), interleave=None):
            for n in n_range:
                ops = outp.tile([128, 256], F32, tag="ops", name=f"op{b}{n}")
                for f in range(ND):
                    def mm(f=f, n=n, ops=ops, b=b):
                        nc.tensor.matmul(
                            ops[:], wo_sb[:, D * f + 128 * n:D * f + 128 * (n + 1)],
                            of[b][:, 256 * f:256 * (f + 1)],
                            start=(f == 0), stop=(f == ND - 1))
                        if f == ND - 1:
                            nc.vector.tensor_scalar_add(
                                osb[b][:, 256 * n:256 * (n + 1)], ops[:],
                                bo_sb[:, n:n + 1])
                    yield mm

        def out_dma(b, eng, half=None):
            rsl = slice(0, D) if half is None else slice(512 * half, 512 * (half + 1))
            csl = slice(0, ND * OWN) if half is None else \
                slice(4 * OWN * half, 4 * OWN * (half + 1))
            nch = ND if half is None else ND // 2
            eng.dma_start(
                out=outT_e[rsl, 256 * b:256 * (b + 1)].rearrange(
                    "(c p) f -> p c f", p=128),
                in_=osb[b][:, csl].rearrange("p (c f) -> p c f", c=nch))

        pend = deque()
        with ExitStack() as attn_scope:
            scp = attn_scope.enter_context(tc.tile_pool(name="sc", bufs=2, space="PSUM"))
            opsp = attn_scope.enter_context(tc.tile_pool(name="ops", bufs=1, space="PSUM"))
            nrmp = attn_scope.enter_context(tc.tile_pool(name="nrm", bufs=4))
            atp = attn_scope.enter_context(tc.tile_pool(name="atp", bufs=BACKLOG + 10))

            # Scope B: attention b0; filler = remaining QKV + wo/x DMAs
            with ExitStack() as phB:
                pspB = phB.enter_context(tc.tile_pool(name="pspB", bufs=1, space="PSUM"))

                def dmas1():
                    nc.sync.dma_start(out=wo_sb[:], in_=wo_e[:])
                    for t in (4, 5):
                        xs = xsp.tile([128, ND * 512], FP16, tag="x", name=f"xst{t}")
                        nc.sync.dma_start(
                            out=xs[:].rearrange("p (c f) -> p c f", c=ND),
                            in_=xT_e[:, 512 * t:512 * (t + 1)]
                                .rearrange("(c p) f -> p c f", p=128))
                        xs_tiles[t] = xs

                def dmas2():
                    for t in (6, 7):
                        xs = xsp.tile([128, ND * 512], FP16, tag="x", name=f"xst{t}")
                        nc.sync.dma_start(
                            out=xs[:].rearrange("p (c f) -> p c f", c=ND),
                            in_=xT_e[:, 512 * t:512 * (t + 1)]
                                .rearrange("(c p) f -> p c f", p=128))
                        xs_tiles[t] = xs

                def scopeB_items():
                    yield from proj_unit_items(pspB, "k", 2, Kt, bk_sb)
                    yield from proj_unit_items(pspB, "k", 3, Kt, bk_sb)
                    yield from proj_unit_items(pspB, "q", 2, Qt, bq_sb)
                    yield from proj_unit_items(pspB, "q", 3, Qt, bq_sb)
                    yield from proj_unit_items(pspB, "v", 1, Vt, bv_sb)
                    yield from vtr_items(trp, 1)
                    yield from proj_unit_items(pspB, "v", 2, Vt, bv_sb)
                    yield from vtr_items(trp, 2)
                    yield from proj_unit_items(pspB, "v", 3, Vt, bv_sb)
                    yield from vtr_items(trp, 3)
                    yield dmas1
                    yield from proj_unit_items(pspB, "k", 4, Kt, bk_sb)
                    yield from proj_unit_items(pspB, "k", 5, Kt, bk_sb)
                    yield from proj_unit_items(pspB, "q", 4, Qt, bq_sb)
                    yield from proj_unit_items(pspB, "q", 5, Qt, bq_sb)
                    yield dmas2
                    yield from proj_unit_items(pspB, "k", 6, Kt, bk_sb)
                    yield from proj_unit_items(pspB, "k", 7, Kt, bk_sb)
                    yield from proj_unit_items(pspB, "q", 6, Qt, bq_sb)
                    yield from proj_unit_items(pspB, "q", 7, Qt, bq_sb)
                    yield from proj_unit_items(pspB, "v", 4, Vt, bv_sb)
                    yield from proj_unit_items(pspB, "v", 5, Vt, bv_sb)
                    yield from proj_unit_items(pspB, "v", 6, Vt, bv_sb)
                    yield from proj_unit_items(pspB, "v", 7, Vt, bv_sb)
                    yield from vtr_items(trp, 4)
                    yield from vtr_items(trp, 5)
                    yield from vtr_items(trp, 6)
                    yield from vtr_items(trp, 7)

                fill_hi.extend(scopeB_items())
                ui = attn_bh(0, 0, scp, opsp, nrmp, atp, pend)
                fill_hi.extendleft(reversed(ui + [rec_dma(0, 0, nc.gpsimd)]))
                ui = attn_bh(0, 1, scp, opsp, nrmp, atp, pend)
                fill_hi.extendleft(reversed(ui + [rec_dma(0, 1, nc.gpsimd)]))
                run_items(pend)   # drain so norms precede leftover transposes
                pend.clear()
                run_items(fill_hi)
                fill_hi.clear()

            # Scope C: attention b1; filler = b0 receive + out-projection
            with ExitStack() as phC:
                outpC = phC.enter_context(tc.tile_pool(name="outpC", bufs=1, space="PSUM"))

                def scopeC_items():
                    g = after_rec(0, 1, 24)
                    for it in outproj_items(0, outpC):
                        it.gate = g
                        yield it

                fill_bg.extend(scopeC_items())
                ui = attn_bh(1, 0, scp, opsp, nrmp, atp, pend)
                fill_hi.extendleft(reversed(ui + [rec_dma(1, 0, nc.gpsimd)]))
                ui = attn_bh(1, 1, scp, opsp, nrmp, atp, pend)
                run_items(pend)   # tail AVs + norm
                pend.clear()
                run_items(ui)     # last OT transposes + ship piece 4
                run_items(fill_hi)
                fill_hi.clear()
                run_items(fill_lo)
                fill_lo.clear()
                run_items(fill_bg)  # outproj-b0 leftovers hide under piece 4
                fill_bg.clear()
                if dbg:
                    nc.sync.dma_start(out=dbg["dQt"][:], in_=Qt[:])
                    nc.sync.dma_start(out=dbg["dKt"][:], in_=Kt[:])
                    nc.sync.dma_start(out=dbg["dVt"][:], in_=Vt[:])
                    nc.sync.dma_start(out=dbg["dOT0"][:], in_=OT[0, 0][:])
                    nc.sync.dma_start(out=dbg["dOT1"][:], in_=OT[0, 1][:])
                    nc.sync.dma_start(out=dbg["dof0"][:], in_=of[0][:])
                out_dma(0, nc.scalar)

            # tail: b1 slot-1 receive + out-projection, interleaved
            attn_scope.close()
            with ExitStack() as phT:
                outpT = phT.enter_context(tc.tile_pool(name="outpT", bufs=2, space="PSUM"))
                rec_dma(1, 1, nc.sync, half=0)()
                rec_dma(1, 1, nc.sync, half=1)()
                run_items(outproj_items(1, outpT, n_range=range(0, 4)))
                out_dma(1, nc.scalar, half=0)
                run_items(outproj_items(1, outpT, n_range=range(4, ND)))
                out_dma(1, nc.scalar, half=1)

    nc.finalize()
    return nc


def _prep_inputs(x, Wq, bq, Wk, bk, Wv, bv, Wo, bo):
    import ml_dtypes
    x = np.ascontiguousarray(np.asarray(x, dtype=np.float32))
    xT = np.ascontiguousarray(x.reshape(T, D).T.astype(np.float16))
    scale = np.float32(1.0 / np.sqrt(DH))
    ident = np.eye(128, dtype=np.float32).astype(ml_dtypes.bfloat16)
    bo_t = np.ascontiguousarray(np.asarray(bo, np.float32).reshape(ND, 128).T)
    wo_bf = (np.asarray(Wo, np.float32).astype(ml_dtypes.bfloat16)
             .reshape(ND, 128, D).transpose(1, 0, 2).reshape(128, ND * D))
    wo_bf = np.ascontiguousarray(wo_bf)

    def pack_w(W, s=1.0):
        W = np.asarray(W, np.float32) * s
        return np.ascontiguousarray(
            W.astype(np.float16).reshape(ND, 128, W.shape[1])
            .transpose(1, 0, 2).reshape(128, ND * W.shape[1]))

    in_maps = []
    for c in range(N_CORES):
        fs = slice(F * c, F * (c + 1))
        in_maps.append({
            "xT": xT,
            "wq": pack_w(np.asarray(Wq, np.float32)[:, fs], scale),
            "wk": pack_w(np.asarray(Wk, np.float32)[:, fs]),
            "wv": pack_w(np.asarray(Wv, np.float32)[:, fs]),
            "bq": np.ascontiguousarray((np.asarray(bq, np.float32)[fs] * scale)[:, None]),
            "bk": np.ascontiguousarray(np.asarray(bk, np.float32)[fs][:, None]),
            "bv": np.ascontiguousarray(np.asarray(bv, np.float32)[fs][:, None]),
            "wo": wo_bf,
            "bo": bo_t,
            "ident": ident,
        })
    return in_maps


def kernel(x, Wq, bq, Wk, bk, Wv, bv, Wo, bo, _trace=False, _trace_kwargs=None):
    if "nc" not in _cache:
        _cache["nc"] = build_nc()
    nc = _cache["nc"]
    in_maps = _prep_inputs(x, Wq, bq, Wk, bk, Wv, bv, Wo, bo)
    res = run_bass_kernel_spmd(nc, in_maps, list(range(N_CORES)),
                               trace=_trace, **(_trace_kwargs or {}))
    _cache["last_results"] = res
    out = np.empty((T, D), np.float32)
    for c in range(N_CORES):
        o = res.results[c]["outT"]  # [D, 2*OWN]
        out[OWN * c:OWN * (c + 1), :] = o[:, 0:OWN].T
        out[S + OWN * c:S + OWN * (c + 1), :] = o[:, OWN:2 * OWN].T
    return out.reshape(B, S, D)


# revision 43
# speedup vs baseline: 1.0023x; 1.0023x over previous
"""Multi-head attention (B=2, S=2048, D=1024, H=16) on 8 TRN2 NeuronCores.

Sharding: tensor-parallel over heads. Core c owns heads {2c, 2c+1}:
  - Q/K/V projections for its 128 feature columns (fp16 weights/x, fp32r Q/K),
  - attention for its 2 heads over both batches; exp'd scores (bf16) act as
    the matmul *stationary* so attention-output lands token-major with the
    softmax denominator accumulated via a ones-column of V,
  - four AllToAll pieces (one per batch x head-slot, bf16, 256KB) convert
    head-sharding -> token-sharding; each core owns 256 tokens of EACH batch
    so batch-0 output projection overlaps batch-1 attention,
  - output projection (full Wo, bf16) per batch for its token slices.
The Activation engine (exp of 16.8M scores/core) is the critical resource:
emission keeps its score->exp pipeline fed from ~16us on, while AV matmuls,
projections for later tiles, V transposes, receive transposes and the
out-projection all flow through a slot-budgeted filler/pend scheduler into
the Act-paced gaps of the strictly in-order PE stream.
Host only reshapes/transposes/concatenates.
"""
import sys
sys.path.insert(0, "/opt/trn_rl_repo")
from collections import deque
from contextlib import ExitStack

import numpy as np

import concourse.bass as bass
import concourse.bacc as bacc
import concourse.mybir as mybir
import concourse.tile as tile
from concourse.bass_utils import run_bass_kernel_spmd

N_CORES = 8
B, S, D = 2, 2048, 1024
T = B * S              # 4096 flattened tokens
H, DH = 16, 64
F = D // N_CORES       # 128 feature columns per core (2 heads)
ND = D // 128          # 8 contraction chunks
NKT = S // 128         # 16 key tiles per batch
OWN = S // N_CORES     # 256 tokens owned per batch per core
BACKLOG = 8            # target AV-lag (iterations) behind the score stream
LAG_MIN = 2            # never let AV catch up closer than this

F32 = mybir.dt.float32
F32R = mybir.dt.float32r
BF16 = mybir.dt.bfloat16
FP16 = mybir.dt.float16
EXP = mybir.ActivationFunctionType.Exp

_cache = {}


def build_nc():
    nc = bacc.Bacc()
    xT_e = nc.dram_tensor("xT", [D, T], FP16, kind="ExternalInput")
    wq_e = nc.dram_tensor("wq", [128, D], FP16, kind="ExternalInput")
    wk_e = nc.dram_tensor("wk", [128, D], FP16, kind="ExternalInput")
    wv_e = nc.dram_tensor("wv", [128, D], FP16, kind="ExternalInput")
    bq_e = nc.dram_tensor("bq", [F, 1], F32, kind="ExternalInput")
    bk_e = nc.dram_tensor("bk", [F, 1], F32, kind="ExternalInput")
    bv_e = nc.dram_tensor("bv", [F, 1], F32, kind="ExternalInput")
    wo_e = nc.dram_tensor("wo", [128, ND * D], BF16, kind="ExternalInput")
    bo_e = nc.dram_tensor("bo", [128, ND], F32, kind="ExternalInput")
    id_e = nc.dram_tensor("ident", [128, 128], BF16, kind="ExternalInput")
    outT_e = nc.dram_tensor("outT", [D, 2 * OWN], F32, kind="ExternalOutput")
    dbg = {}
    if _cache.get("debug"):
        for nm, shape, dt in (("dQt", [F, T], F32R), ("dKt", [F, T], F32R),
                              ("dVt", [F, T], BF16), ("dOT0", [128, 1024], BF16),
                              ("dOT1", [128, 1024], BF16),
                              ("dof0", [128, ND * 256], BF16)):
            dbg[nm] = nc.dram_tensor(nm, shape, dt, kind="ExternalOutput")

    with tile.TileContext(nc) as tc, ExitStack() as top:
        # persistent SBUF tensors
        big = top.enter_context(tc.tile_pool(name="big", bufs=1))
        Qt = big.tile([F, T], F32R, tag="Qt")        # [feat, tok]
        Kt = big.tile([F, T], F32R, tag="Kt")
        Vt = big.tile([F, T], BF16, tag="Vt")
        w_sb = {n: big.tile([128, D], FP16, tag=f"w{n}", name=f"w_{n}")
                for n in ("q", "k", "v")}
        wo_sb = big.tile([128, ND * D], BF16, tag="wo")

        misc = top.enter_context(tc.tile_pool(name="misc", bufs=1))
        bq_sb = misc.tile([F, 1], F32)
        bk_sb = misc.tile([F, 1], F32)
        bv_sb = misc.tile([F, 1], F32)
        bo_sb = misc.tile([128, ND], F32)
        id_sb = misc.tile([128, 128], BF16)

        xsp = top.enter_context(tc.tile_pool(name="xst", bufs=4))
        xs_tiles = {}

        # startup DMA order: first x tiles + wk first so K matmuls start ASAP
        for t in range(4):
            xs_tiles[t] = xsp.tile([128, ND * 512], FP16, tag="x", name=f"xst{t}")
        for t, (we, wn, be, bs) in enumerate((
                (wk_e, "k", bk_e, bk_sb), (wq_e, "q", bq_e, bq_sb),
                (wv_e, "v", bv_e, bv_sb), (None, None, bo_e, bo_sb))):
            xs = xs_tiles[t]
            nc.sync.dma_start(
                out=xs[:].rearrange("p (c f) -> p c f", c=ND)[:, :, 0:256],
                in_=xT_e[:, 512 * t:512 * t + 256].rearrange("(c p) f -> p c f", p=128))
            if we is not None:
                nc.scalar.dma_start(out=w_sb[wn][:], in_=we[:])
            nc.scalar.dma_start(
                out=xs[:].rearrange("p (c f) -> p c f", c=ND)[:, :, 256:512],
                in_=xT_e[:, 512 * t + 256:512 * (t + 1)].rearrange("(c p) f -> p c f", p=128))
            nc.sync.dma_start(out=bs[:], in_=be[:])
        nc.sync.dma_start(out=id_sb[:], in_=id_e[:])

        vsb = top.enter_context(tc.tile_pool(name="vsb", bufs=1))
        v_tiles = {}

        # OT[b]: normalized attn out, feat-major: rows 64h:64h+64 = slot h,
        # cols 128*qs + tok for the 16 q-slices
        onp = top.enter_context(tc.tile_pool(name="onp", bufs=1))
        # OT[b,h]: [64*(qs%2)+f, 128*(qs//2)+t] -- row-groups by query parity,
        # columns grouped by owning core
        OT = {(b, h): onp.tile([128, NKT * 64], BF16, tag=f"OT{b}{h}",
                               name=f"OT{b}{h}")
              for b in range(B) for h in range(2)}
        onrmp = top.enter_context(tc.tile_pool(name="onrm", bufs=20))

        dram = top.enter_context(tc.tile_pool(name="dram", bufs=1, space="DRAM"))
        pin = {}
        pout = {}
        for b in range(B):
            for h in range(2):
                pin[b, h] = dram.tile([N_CORES, DH, OWN], BF16, tag=f"pi{b}{h}",
                                      name=f"pi{b}{h}")
                pout[b, h] = dram.tile([N_CORES, DH, OWN], BF16, tag=f"po{b}{h}",
                                       name=f"po{b}{h}")

        ofp = top.enter_context(tc.tile_pool(name="ofp", bufs=1))
        of = {b: ofp.tile([128, ND * 256], BF16, tag=f"of{b}", name=f"of{b}")
              for b in range(B)}
        osbp = top.enter_context(tc.tile_pool(name="osbp", bufs=1))
        osb = {b: osbp.tile([128, ND * OWN], F32, tag=f"osb{b}", name=f"osb{b}")
               for b in range(B)}

        # shared 1-bank transpose pool, alive for the whole kernel
        trp = top.enter_context(tc.tile_pool(name="trp", bufs=1, space="PSUM"))

        # --- emission helpers: items are closures (one PE instruction each,
        # DVE followups attached), consumed via the slot scheduler ---
        def proj_unit_items(psp, proj, t, dst, bias_sb):
            cell = []
            for dk in range(ND):
                def mm(dk=dk, cell=cell, proj=proj, t=t, psp=psp,
                       dst=dst, bias_sb=bias_sb):
                    if dk == 0:
                        cell.append(psp.tile([128, 512], F32, tag="pj",
                                             name=f"pj_{proj}{t}"))
                    ps = cell[0]
                    nc.tensor.matmul(ps[:], w_sb[proj][:, 128 * dk:128 * (dk + 1)],
                                     xs_tiles[t][:, 512 * dk:512 * (dk + 1)],
                                     start=(dk == 0), stop=(dk == ND - 1))
                    if dk == ND - 1:
                        sl = slice(512 * t, 512 * (t + 1))
                        nc.vector.tensor_scalar_add(dst[:, sl], ps[:], bias_sb[:])
                yield mm

        def vtr_items(pool, t):
            b = t // 4
            for i in range(4):
                kt = 4 * (t % 4) + i
                tok = 512 * t + 128 * i
                tp = pool.tile([128, 128], BF16, tag="tr", name=f"tr{t}{i}")

                def tr(tp=tp, tok=tok, b=b, kt=kt):
                    nc.tensor.transpose(tp[:], Vt[:, tok:tok + 128], id_sb[:])
                    for h in range(2):
                        vt = vsb.tile([128, 65], BF16, tag=f"v{b}{h}{kt}",
                                      name=f"v{b}{h}{kt}")
                        nc.vector.tensor_copy(vt[:, 0:64], tp[:, 64 * h:64 * (h + 1)])
                        nc.vector.memset(vt[:, 64:65], 1.0)
                        v_tiles[b, h, kt] = vt
                yield tr

        def run_items(items):
            for it in items:
                it()

        # ---- upfront: K(t0,t1), Q(t0,t1), V(t0)+vtr(t0) ----
        with ExitStack() as phA:
            pspA = phA.enter_context(tc.tile_pool(name="pspA", bufs=2, space="PSUM"))
            for t in range(2):
                run_items(proj_unit_items(pspA, "k", t, Kt, bk_sb))
            for t in range(2):
                run_items(proj_unit_items(pspA, "q", t, Qt, bq_sb))
            run_items(proj_unit_items(pspA, "v", 0, Vt, bv_sb))
            run_items(vtr_items(trp, 0))

        # ---- attention unit (b, h): scores/exp stream paces Act; AV, norm,
        # ship and filler flow through the pend/filler slot scheduler ----
        def emittable(fn):
            key = getattr(fn, "vt_key", None)
            return key is None or key in v_tiles

        def gated(fn):
            g = getattr(fn, "gate", None)
            return g is None or g()

        def attn_bh(b, h, scp, opsp, nrmp, atp, pend):
            unit_items = []
            hs = slice(64 * h, 64 * (h + 1))
            for qh in range(2):
                o_ps = [opsp.tile([128, 512], F32, tag=f"o{i}", bufs=1,
                                  name=f"ops{b}{h}{qh}{i}") for i in range(2)]

                for kt in range(NKT):
                    ktok = 2048 * b + 128 * kt
                    sc = scp.tile([128, 1024], F32, tag="sc",
                                  name=f"sc{b}{h}{qh}{kt}")
                    for i in range(2):
                        qtok = 2048 * b + 1024 * qh + 512 * i
                        nc.tensor.matmul(sc[:, 512 * i:512 * (i + 1)],
                                         Kt[hs, ktok:ktok + 128],
                                         Qt[hs, qtok:qtok + 512],
                                         start=True, stop=True)
                    at = atp.tile([128, 1024], BF16, tag="at",
                                  name=f"at{b}{h}{qh}{kt}")
                    nc.scalar.activation(at[:], sc[:], EXP)

                    def avs(kt=kt, at=at, o_ps=o_ps, b=b, h=h):
                        vt = v_tiles[b, h, kt]
                        for j8 in range(8):
                            ti, j = j8 // 4, j8 % 4
                            # HW start=True zeroes the WHOLE PSUM bank, so
                            # only slot j==0 of each bank starts; the other
                            # slots accumulate onto the zeroed bank
                            nc.tensor.matmul(o_ps[ti][:, 128 * j:128 * j + 65],
                                             at[:, 128 * j8:128 * (j8 + 1)],
                                             vt[:, 0:65],
                                             start=(kt == 0 and j == 0),
                                             stop=(kt == NKT - 1),
                                             skip_group_check=True)
                    avs.vt_key = (b, h, kt)
                    pend.append(avs)
                    it_ctr[0] += 1
                    # non-PE pend items (norm/ship/rec-DMA) pop for free
                    while pend and getattr(pend[0], "free", False):
                        pend.popleft()()
                    # slot budget: ~3 x 213ns of PE work besides the scores
                    spent = 0
                    while spent < 2 and len(pend) > BACKLOG and emittable(pend[0]):
                        it = pend.popleft()
                        spent += getattr(it, "slots", 1)
                        it()
                    while spent < 3:
                        if fill_hi and gated(fill_hi[0]):
                            it = fill_hi.popleft()
                        elif fill_lo and gated(fill_lo[0]):
                            it = fill_lo.popleft()
                        elif fill_bg and gated(fill_bg[0]):
                            it = fill_bg.popleft()
                        else:
                            break
                        spent += getattr(it, "slots", 1)
                        it()
                    while spent < 3 and len(pend) > LAG_MIN and emittable(pend[0]):
                        it = pend.popleft()
                        spent += getattr(it, "slots", 1)
                        it()

                onrm = {}

                def norm(o_ps=o_ps, qh=qh, b=b, h=h, onrm=onrm):
                    for ti in range(2):
                        rcp = nrmp.tile([128, 4], F32, tag="rcp",
                                        name=f"rcp{b}{h}{qh}{ti}")
                        den = o_ps[ti][:].rearrange("p (j c) -> p j c", c=128)[:, :, 64:65]
                        nc.vector.reciprocal(rcp[:].rearrange("p (j c) -> p j c", c=1), den)
                        for j in range(4):
                            qs = 8 * qh + 4 * ti + j
                            pair = (qs % 8) // 2
                            if pair not in onrm:
                                onrm[pair] = onrmp.tile([128, 128], BF16, tag="onrm",
                                                        name=f"onrm{b}{h}{qh}{pair}")
                            nc.vector.tensor_scalar_mul(
                                onrm[pair][:, 64 * (qs % 2):64 * (qs % 2 + 1)],
                                o_ps[ti][:, 128 * j:128 * j + 64],
                                rcp[:, j:j + 1])
                norm.free = True
                pend.append(norm)
                for pair in range(4):
                    def trc(pair=pair, qh=qh, b=b, h=h, onrm=onrm):
                        pg = 4 * qh + pair
                        tp = trp.tile([128, 128], BF16, tag="tr",
                                      name=f"otr{b}{h}{pg}")
                        nc.tensor.transpose(tp[:], onrm[pair][:], id_sb[:])
                        nc.vector.tensor_copy(
                            OT[b, h][:, 128 * pg:128 * (pg + 1)], tp[:])
                    trc.slots = 2
                    trc.gate = (lambda pair=pair, onrm=onrm: pair in onrm)
                    unit_items.append(trc)

            def ship(b=b, h=h):
                for j in range(2):
                    nc.sync.dma_start(
                        out=pin[b, h][:, :, 128 * j:128 * (j + 1)]
                        .rearrange("r f t -> f r t"),
                        in_=OT[b, h][64 * j:64 * (j + 1), :]
                        .rearrange("f (r t) -> f r t", r=N_CORES))
                nc.gpsimd.collective_compute(
                    "AllToAll", mybir.AluOpType.bypass,
                    ins=[pin[b, h][:].opt()], outs=[pout[b, h][:].opt()],
                    replica_groups=[list(range(N_CORES))])
            unit_items.append(ship)
            return unit_items

        rec_flags = {}
        it_ctr = [0]
        fill_hi = deque()
        fill_lo = deque()
        fill_bg = deque()

        def rec_dma(b, h, eng, half=None):
            def rdma(b=b, h=h, eng=eng, half=half):
                rsl = slice(0, N_CORES) if half is None else \
                    slice(4 * half, 4 * (half + 1))
                eng.dma_start(
                    out=of[b][64 * h:64 * (h + 1), :].rearrange(
                        "f (r t) -> f r t", r=N_CORES)[:, rsl, :],
                    in_=pout[b, h][rsl].rearrange("r f t -> f r t"),
                )
                rec_flags[b, h] = it_ctr[0]
            return rdma

        def after_rec(b, h, margin):
            """Gate: open `margin` iterations after the (b,h) receive DMA was
            dispatched (collective + transfer latency, in ~1.05us iterations)."""
            def g():
                it = rec_flags.get((b, h))
                return it is not None and it_ctr[0] >= it + margin
            return g


        def outproj_items(b, outp, n_range=range(ND), interleave=None):
            for n in n_range:
                ops = outp.tile([128, 256], F32, tag="ops", name=f"op{b}{n}")
                for f in range(ND):
                    def mm(f=f, n=n, ops=ops, b=b):
                        nc.tensor.matmul(
                            ops[:], wo_sb[:, D * f + 128 * n:D * f + 128 * (n + 1)],
                            of[b][:, 256 * f:256 * (f + 1)],
                            start=(f == 0), stop=(f == ND - 1))
                        if f == ND - 1:
                            nc.vector.tensor_scalar_add(
                                osb[b][:, 256 * n:256 * (n + 1)], ops[:],
                                bo_sb[:, n:n + 1])
                    yield mm

        def out_dma(b, eng, half=None):
            rsl = slice(0, D) if half is None else slice(512 * half, 512 * (half + 1))
            csl = slice(0, ND * OWN) if half is None else \
                slice(4 * OWN * half, 4 * OWN * (half + 1))
            nch = ND if half is None else ND // 2
            eng.dma_start(
                out=outT_e[rsl, 256 * b:256 * (b + 1)].rearrange(
                    "(c p) f -> p c f", p=128),
                in_=osb[b][:, csl].rearrange("p (c f) -> p c f", c=nch))

        pend = deque()
        with ExitStack() as attn_scope:
            scp = attn_scope.enter_context(tc.tile_pool(name="sc", bufs=2, space="PSUM"))
            opsp = attn_scope.enter_context(tc.tile_pool(name="ops", bufs=1, space="PSUM"))
            nrmp = attn_scope.enter_context(tc.tile_pool(name="nrm", bufs=4))
            atp = attn_scope.enter_context(tc.tile_pool(name="atp", bufs=BACKLOG + 10))

            # Scope B: attention b0; filler = remaining QKV + wo/x DMAs
            with ExitStack() as phB:
                pspB = phB.enter_context(tc.tile_pool(name="pspB", bufs=1, space="PSUM"))

                def dmas1():
                    nc.sync.dma_start(out=wo_sb[:], in_=wo_e[:])
                    for t in (4, 5):
                        xs = xsp.tile([128, ND * 512], FP16, tag="x", name=f"xst{t}")
                        nc.sync.dma_start(
                            out=xs[:].rearrange("p (c f) -> p c f", c=ND),
                            in_=xT_e[:, 512 * t:512 * (t + 1)]
                                .rearrange("(c p) f -> p c f", p=128))
                        xs_tiles[t] = xs

                def dmas2():
                    for t in (6, 7):
                        xs = xsp.tile([128, ND * 512], FP16, tag="x", name=f"xst{t}")
                        nc.sync.dma_start(
                            out=xs[:].rearrange("p (c f) -> p c f", c=ND),
                            in_=xT_e[:, 512 * t:512 * (t + 1)]
                                .rearrange("(c p) f -> p c f", p=128))
                        xs_tiles[t] = xs

                def scopeB_items():
                    yield from proj_unit_items(pspB, "k", 2, Kt, bk_sb)
                    yield from proj_unit_items(pspB, "k", 3, Kt, bk_sb)
                    yield from proj_unit_items(pspB, "q", 2, Qt, bq_sb)
                    yield from proj_unit_items(pspB, "q", 3, Qt, bq_sb)
                    yield from proj_unit_items(pspB, "v", 1, Vt, bv_sb)
                    yield from vtr_items(trp, 1)
                    yield from proj_unit_items(pspB, "v", 2, Vt, bv_sb)
                    yield from vtr_items(trp, 2)
                    yield from proj_unit_items(pspB, "v", 3, Vt, bv_sb)
                    yield from vtr_items(trp, 3)
                    yield dmas1
                    yield from proj_unit_items(pspB, "k", 4, Kt, bk_sb)
                    yield from proj_unit_items(pspB, "k", 5, Kt, bk_sb)
                    yield from proj_unit_items(pspB, "q", 4, Qt, bq_sb)
                    yield from proj_unit_items(pspB, "q", 5, Qt, bq_sb)
                    yield dmas2
                    yield from proj_unit_items(pspB, "k", 6, Kt, bk_sb)
                    yield from proj_unit_items(pspB, "k", 7, Kt, bk_sb)
                    yield from proj_unit_items(pspB, "q", 6, Qt, bq_sb)
                    yield from proj_unit_items(pspB, "q", 7, Qt, bq_sb)
                    yield from proj_unit_items(pspB, "v", 4, Vt, bv_sb)
                    yield from proj_unit_items(pspB, "v", 5, Vt, bv_sb)
                    yield from proj_unit_items(pspB, "v", 6, Vt, bv_sb)
                    yield from proj_unit_items(pspB, "v", 7, Vt, bv_sb)
                    yield from vtr_items(trp, 4)
                    yield from vtr_items(trp, 5)
                    yield from vtr_items(trp, 6)
                    yield from vtr_items(trp, 7)

                fill_hi.extend(scopeB_items())
                ui = attn_bh(0, 0, scp, opsp, nrmp, atp, pend)
                fill_hi.extendleft(reversed(ui + [rec_dma(0, 0, nc.gpsimd)]))
                ui = attn_bh(0, 1, scp, opsp, nrmp, atp, pend)
                fill_hi.extendleft(reversed(ui + [rec_dma(0, 1, nc.gpsimd)]))
                run_items(pend)   # drain so norms precede leftover transposes
                pend.clear()
                run_items(fill_hi)
                fill_hi.clear()

            # Scope C: attention b1; filler = b0 receive + out-projection
            with ExitStack() as phC:
                outpC = phC.enter_context(tc.tile_pool(name="outpC", bufs=1, space="PSUM"))

                def scopeC_items():
                    g = after_rec(0, 1, 24)
                    for it in outproj_items(0, outpC):
                        it.gate = g
                        yield it

                fill_bg.extend(scopeC_items())
                ui = attn_bh(1, 0, scp, opsp, nrmp, atp, pend)
                fill_hi.extendleft(reversed(ui + [rec_dma(1, 0, nc.gpsimd)]))
                ui = attn_bh(1, 1, scp, opsp, nrmp, atp, pend)
                run_items(pend)   # tail AVs + norm
                pend.clear()
                run_items(ui)     # last OT transposes + ship piece 4
                run_items(fill_hi)
                fill_hi.clear()
                run_items(fill_lo)
                fill_lo.clear()
                run_items(fill_bg)  # outproj-b0 leftovers hide under piece 4
                fill_bg.clear()
                if dbg:
                    nc.sync.dma_start(out=dbg["dQt"][:], in_=Qt[:])
                    nc.sync.dma_start(out=dbg["dKt"][:], in_=Kt[:])
                    nc.sync.dma_start(out=dbg["dVt"][:], in_=Vt[:])
                    nc.sync.dma_start(out=dbg["dOT0"][:], in_=OT[0, 0][:])
                    nc.sync.dma_start(out=dbg["dOT1"][:], in_=OT[0, 1][:])
                    nc.sync.dma_start(out=dbg["dof0"][:], in_=of[0][:])
                out_dma(0, nc.scalar)

            # tail: b1 slot-1 receive + out-projection, interleaved
            attn_scope.close()
            with ExitStack() as phT:
                outpT = phT.enter_context(tc.tile_pool(name="outpT", bufs=2, space="PSUM"))
                rec_dma(1, 1, nc.sync, half=0)()
                rec_dma(1, 1, nc.sync, half=1)()
                run_items(outproj_items(1, outpT, n_range=range(0, 4)))
                out_dma(1, nc.scalar, half=0)
                run_items(outproj_items(1, outpT, n_range=range(4, ND)))
                out_dma(1, nc.scalar, half=1)

    nc.finalize()
    return nc


def _prep_inputs(x, Wq, bq, Wk, bk, Wv, bv, Wo, bo):
    import ml_dtypes
    x = np.ascontiguousarray(np.asarray(x, dtype=np.float32))
    xT = np.ascontiguousarray(x.reshape(T, D).T.astype(np.float16))
    scale = np.float32(1.0 / np.sqrt(DH))
    ident = np.eye(128, dtype=np.float32).astype(ml_dtypes.bfloat16)
    bo_t = np.ascontiguousarray(np.asarray(bo, np.float32).reshape(ND, 128).T)
    wo_bf = (np.asarray(Wo, np.float32).astype(ml_dtypes.bfloat16)
             .reshape(ND, 128, D).transpose(1, 0, 2).reshape(128, ND * D))
    wo_bf = np.ascontiguousarray(wo_bf)

    def pack_w(W, s=1.0):
        W = np.asarray(W, np.float32) * s
        return np.ascontiguousarray(
            W.astype(np.float16).reshape(ND, 128, W.shape[1])
            .transpose(1, 0, 2).reshape(128, ND * W.shape[1]))

    in_maps = []
    for c in range(N_CORES):
        fs = slice(F * c, F * (c + 1))
        in_maps.append({
            "xT": xT,
            "wq": pack_w(np.asarray(Wq, np.float32)[:, fs], scale),
            "wk": pack_w(np.asarray(Wk, np.float32)[:, fs]),
            "wv": pack_w(np.asarray(Wv, np.float32)[:, fs]),
            "bq": np.ascontiguousarray((np.asarray(bq, np.float32)[fs] * scale)[:, None]),
            "bk": np.ascontiguousarray(np.asarray(bk, np.float32)[fs][:, None]),
            "bv": np.ascontiguousarray(np.asarray(bv, np.float32)[fs][:, None]),
            "wo": wo_bf,
            "bo": bo_t,
            "ident": ident,
        })
    return in_maps


def kernel(x, Wq, bq, Wk, bk, Wv, bv, Wo, bo, _trace=False, _trace_kwargs=None):
    if "nc" not in _cache:
        _cache["nc"] = build_nc()
    nc = _cache["nc"]
    in_maps = _prep_inputs(x, Wq, bq, Wk, bk, Wv, bv, Wo, bo)
    res = run_bass_kernel_spmd(nc, in_maps, list(range(N_CORES)),
                               trace=_trace, **(_trace_kwargs or {}))
    _cache["last_results"] = res
    out = np.empty((T, D), np.float32)
    for c in range(N_CORES):
        o = res.results[c]["outT"]  # [D, 2*OWN]
        out[OWN * c:OWN * (c + 1), :] = o[:, 0:OWN].T
        out[S + OWN * c:S + OWN * (c + 1), :] = o[:, OWN:2 * OWN].T
    return out.reshape(B, S, D)
